# revision 31
# baseline (speedup 1.0000x reference)
"""Local2DAttention TRN2 kernel (nn_Local2DAttention_79207786873330).

Math (faithful to the reference's torch-bug semantics):
  x (16, 1024, 512) is window-blocked into M=256 "windows" (b, i, j) of 8x8
  spatial positions. A plain row-major reshape of each (E, 8, 8) block into
  (64, 512) scrambles channels/spatial into 64 tokens per window:
      y[m, t, e] = x[b, (i*8+w1)*32 + j*8 + w2, 8t+a],  e = a*64 + w1*8 + w2
  nn.MultiheadAttention (batch_first=False) then attends over the M=256 axis
  with the 64 t-positions as batch and 8 heads:
      per (t, h): S = Q K^T / 8 over 256x256, softmax, O = P V.

Sharding: the 64 t-positions split 8 per core (t = 8*cc + tl). Attention,
projections and output assembly are fully independent per t -> zero
cross-core communication. Weights are replicated.

Device pipeline per core (bf16 matmul operands, fp32 accumulation):
  yT (512, 2048)      - host-permuted token matrix, f-major (e x tokens)
  QK^T proj: PROJ^T[r, tok] = Wqk^T.T @ yT  (q rows pre-scaled by 1/8),
             bias-evicts split ACT (Identity + bias AP) / DVE (tensor_scalar)
  V    proj: V'[tok, 8x66]  = yT.T @ Wv''   (66-stride heads, ones col at 64)
  per (t, h):  S^T = K^T.T @ Q^T  - head pairs at tile_position (0,0)/(64,0)
               emitted row-group-alternating so they run CONCURRENTLY on the
               PE's 32x32 subarrays; exp on ACT (->bf16)
               AV swapped: stationary = P^T chunk, moving = V'[:, 65 cols]
                 -> O_u (q on PARTITIONS, 64 hd + D in col 64), 4 heads
                    packed per PSUM tile (128, 260)
               normalize: DVE reciprocal (128,4) + broadcast multiply
                 (per-partition D -> no PE broadcast matmul, no 1-lane recip)
  transpose O (PE, vs identity) -> O^T chunks -> evict to SBUF (DVE)
  out proj: Z = O^T.T @ Wout^T; Z += b_eff (DVE); DMA from SBUF (sync queue)

Schedule (the key to the 3.6x over the first working version):
  - per t-pair: QK r-tiles interleaved with S(t0) head-pairs, so t0's exps
    stream on ACT across the projection phase; S(t1) head-pairs spread
    between the AV/T/O blocks of the tail for the same reason. ACT is the
    attention-phase pacer - everything else must keep out of its way.
  - S-score PSUM tiles live in a dedicated 4-bank pool: main-pool rotation
    must never make a PE matmul wait on the ACT exp queue.
  - input DMAs: few large chunks with >=1KB per-partition runs (256-B runs
    measured ~4x slower), critical-first ordering, balanced across the
    gpsimd/sync/scalar queues; outputs all on sync (scalar triggers block
    exps in ACT's FIFO; gpsimd's end-drain with in-flight DMA costs ~3us).
"""
import sys
sys.path.insert(0, '/opt/trn_rl_repo')
import numpy as np
import ml_dtypes

BF = ml_dtypes.bfloat16

# problem constants (hardcoded per contract)
B, N, E = 16, 1024, 512
WIN = 8          # window_size
HS = 4           # hS = S // W,  S = 32
NH = 8           # heads
HD = 64          # head dim
NCORES = 8
TL = 8           # t-values per core
MTOK = 256       # windows (= B*HS*HS) = tokens per t
TOK = TL * MTOK  # tokens per core

_cache = {}


def _split_multiwaits(nc, mybir, limit=1):
    """This toolchain's walrus encodes at most one semaphore wait per
    instruction; hoist excess waits into preceding NoOps on the same engine."""
    n_split = 0
    for f in nc.m.functions:
        for blk in f.blocks:
            insts = blk.instructions
            out = []
            for inst in insts:
                si = inst.sync_info
                waits = list(si.on_wait) if (si is not None and si.on_wait) else []
                if len(waits) > limit:
                    excess, keep = waits[:-limit], waits[-limit:]
                    for w in excess:
                        nop = mybir.InstNoOp(
                            name=f"{inst.name}-wsplit{n_split}",
                            engine=inst.engine,
                            ins=[], outs=[],
                            sync_info=mybir.SyncInfo(on_wait=[w], on_update=[]),
                        )
                        out.append(nop)
                        n_split += 1
                    inst.sync_info = mybir.SyncInfo(
                        on_wait=keep, on_update=list(si.on_update or []))
                out.append(inst)
            if n_split:
                insts.clear()
                insts.extend(out)
    return n_split


def _build_module(split_waits=True):
    import concourse.bass as bass
    import concourse.mybir as mybir
    from concourse import tile

    f32 = mybir.dt.float32
    bf16 = mybir.dt.bfloat16
    Exp = mybir.ActivationFunctionType.Exp
    Ident = mybir.ActivationFunctionType.Identity

    nc = bass.Bass()
    YT = nc.dram_tensor("yT", [E, TOK], bf16, kind="ExternalInput")
    WQK = nc.dram_tensor("wqk", [E, 2 * E], bf16, kind="ExternalInput")
    WV = nc.dram_tensor("wv", [E, NH * 66], bf16, kind="ExternalInput")
    WO = nc.dram_tensor("wo", [E, E], bf16, kind="ExternalInput")
    BQK = nc.dram_tensor("bqk", [128, 8], f32, kind="ExternalInput")
    BEFF = nc.dram_tensor("beff", [1, E], f32, kind="ExternalInput")
    IDENT = nc.dram_tensor("ident", [128, 128], bf16, kind="ExternalInput")
    OUT = nc.dram_tensor("o", [TOK, E], f32, kind="ExternalOutput")

    with tile.TileContext(nc) as tc:
        with (
            tc.tile_pool(name="persist", bufs=1) as pers,
            tc.tile_pool(name="qk", bufs=2) as qkp,
            tc.tile_pool(name="v", bufs=2) as vpool,
            tc.tile_pool(name="pt", bufs=2) as ptp,
            tc.tile_pool(name="on", bufs=3) as onp,
            tc.tile_pool(name="ot", bufs=2) as otp,
            tc.tile_pool(name="sm", bufs=4) as smp,
            tc.tile_pool(name="z", bufs=3) as zp,
            tc.tile_pool(name="ps", bufs=4, space="PSUM") as psp,
            tc.tile_pool(name="psS", bufs=4, space="PSUM") as psSp,
        ):
            # ---- persistent loads -------------------------------------
            # Inputs live in single coalesced SBUF tiles (col-blocked by the
            # 128-row E chunk fi) so each load is ONE multi-block DMA -- the
            # head was trigger-serialization-bound with per-(fi,chunk) DMAs.
            # Order: the r-tile pairs the first S head-pairs need, and the
            # tp0 token slice, land first.
            bqks = pers.tile([128, 8], f32, tag="bqk")
            # fi-pair chunks keep per-partition runs at 1 KB (256-B runs of a
            # 4-block AP measured ~4x slower); Q halves land before K halves
            wqkall = pers.tile([128, 4 * 2 * E], bf16, tag="wqkall")
            wqk4 = wqkall[:].rearrange("p (f c) -> p f c", f=4)
            ytall = pers.tile([128, 4 * TOK], bf16, tag="ytall")
            yt4 = ytall[:].rearrange("p (f c) -> p f c", f=4)

            def load_wqk(rp, fp, eng):
                # quarter chunks (r-tile pair x fi pair, 128 KB) so the first
                # r-tiles are usable after ~128 KB per queue
                eng.dma_start(
                    wqk4[:, 2 * fp:2 * fp + 2, rp * 256:(rp + 1) * 256],
                    WQK[fp * 256:(fp + 1) * 256,
                        rp * 256:(rp + 1) * 256].rearrange(
                        "(f p) c -> p f c", f=2))

            def load_yt(tp, fp, eng):
                eng.dma_start(
                    yt4[:, 2 * fp:2 * fp + 2, tp * 512:(tp + 1) * 512],
                    YT[fp * 256:(fp + 1) * 256,
                       tp * 512:(tp + 1) * 512].rearrange(
                        "(f p) c -> p f c", f=2))

            # queue order tuned so no queue carries more than ~256 KB of the
            # first QK group's operands (r-tiles 0-1 + yt tp0); later
            # r-tiles arrive behind compute
            load_wqk(0, 0, nc.gpsimd)   # r-tiles 0,1
            load_wqk(0, 1, nc.sync)
            load_yt(0, 1, nc.scalar)
            nc.gpsimd.dma_start(        # yt tp0 fi=0
                ytall[:, 0:512], YT[0:128, 0:512])
            nc.sync.dma_start(          # yt tp0 fi=1
                ytall[:, TOK:TOK + 512], YT[128:256, 0:512])
            nc.sync.dma_start(bqks[:], BQK[:])
            load_wqk(1, 1, nc.sync)     # r-tiles 2,3
            load_wqk(1, 0, nc.gpsimd)
            load_wqk(2, 1, nc.sync)     # r-tiles 4,5
            load_wqk(2, 0, nc.gpsimd)
            load_yt(1, 1, nc.scalar)
            load_wqk(3, 1, nc.sync)     # r-tiles 6,7
            load_wqk(3, 0, nc.gpsimd)
            load_yt(1, 0, nc.sync)
            for tp in (2, 3):
                load_yt(tp, 0, nc.sync)
                load_yt(tp, 1, nc.scalar)
            wvall = pers.tile([128, 4 * NH * 66], bf16, tag="wvall")
            wv4 = wvall[:].rearrange("p (f c) -> p f c", f=4)
            for fp in range(2):
                nc.gpsimd.dma_start(
                    wv4[:, 2 * fp:2 * fp + 2, :],
                    WV[fp * 256:(fp + 1) * 256, :].rearrange(
                        "(f p) c -> p f c", f=2))
            woall = pers.tile([128, 4 * E], bf16, tag="woall")
            wo4 = woall[:].rearrange("p (f c) -> p f c", f=4)
            for fp in range(2):
                nc.gpsimd.dma_start(
                    wo4[:, 2 * fp:2 * fp + 2, :],
                    WO[fp * 256:(fp + 1) * 256, :].rearrange(
                        "(f p) c -> p f c", f=2))
            beffb = pers.tile([128, E], f32, tag="beffb")
            nc.scalar.dma_start(
                beffb[:], BEFF[:].partition_broadcast(128).squeeze(1))
            ident = pers.tile([128, 128], bf16, tag="ident")
            nc.scalar.dma_start(ident[:], IDENT[:])



            def emit_transpose(t, otns):
                """O (q-part, E-free) -> O^T (E-part, m-free) via PE.
                qc-major bank packing: the out-proj of qc only needs its own
                bank's evict, so O(qc0) overlaps the qc1 transposes."""
                otT = otp.tile([128, 1024], bf16, tag="otT", name=f"otT_{t}")
                for qc in range(2):
                    psT = psp.tile([128, 512], bf16, tag="ps",
                                   name=f"psT_{t}_{qc}")
                    for ec in range(4):
                        nc.tensor.matmul(
                            psT[:, ec * 128:(ec + 1) * 128],
                            otns[qc][:, ec * 128:(ec + 1) * 128],
                            ident[:], is_transpose=True,
                            start=True, stop=True, skip_group_check=True)
                    nc.vector.tensor_copy(
                        otT[:, qc * 512:(qc + 1) * 512], psT[:])
                return otT

            def emit_outproj(t, otT, last=False):
                tok0 = t * MTOK
                for qc in range(2):
                    if last and qc == 1:
                        # final store: column-half accumulation groups so the
                        # DVE add + DMA of half 0 overlap half 1's matmuls;
                        # the two stores go on different queues (ACT's FIFO
                        # is empty at drain time, so scalar is safe here)
                        zt = zp.tile([128, E], f32, tag="zt",
                                     name=f"zt_{t}_{qc}")
                        for ch in range(2):
                            pszh = psp.tile([128, 256], f32, tag="ps",
                                            name=f"pszh_{t}_{ch}")
                            for ec in range(4):
                                nc.tensor.matmul(
                                    pszh[:],
                                    otT[:, qc * 512 + ec * 128:
                                        qc * 512 + (ec + 1) * 128],
                                    woall[:, ec * E + ch * 256:
                                           ec * E + (ch + 1) * 256],
                                    start=(ec == 0), stop=(ec == 3))
                            nc.vector.tensor_add(
                                zt[:, ch * 256:(ch + 1) * 256], pszh[:],
                                beffb[:, ch * 256:(ch + 1) * 256])
                            eng = nc.sync if ch == 0 else nc.scalar
                            eng.dma_start(
                                OUT[tok0 + qc * 128:tok0 + (qc + 1) * 128,
                                    ch * 256:(ch + 1) * 256],
                                zt[:, ch * 256:(ch + 1) * 256])
                        continue
                    psz = psp.tile([128, E], f32, tag="ps",
                                   name=f"psz_{t}_{qc}")
                    for ec in range(4):
                        nc.tensor.matmul(
                            psz[:],
                            otT[:, qc * 512 + ec * 128:qc * 512 + (ec + 1) * 128],
                            woall[:, ec * E:(ec + 1) * E],
                            start=(ec == 0), stop=(ec == 3))
                    zt = zp.tile([128, E], f32, tag="zt", name=f"zt_{t}_{qc}")
                    # all outputs on sync: scalar triggers would block exps in
                    # ACT's FIFO, and a gpsimd queue with in-flight DMA costs
                    # ~3us in the end-of-kernel drain
                    nc.vector.tensor_add(zt[:], psz[:], beffb[:])
                    nc.sync.dma_start(
                        OUT[tok0 + qc * 128:tok0 + (qc + 1) * 128, :],
                        zt[:])

            def emit_av(t, ti, pts, vts, otns, hg):
                """AV (swapped) + normalize for one head-group of t, both
                query chunks. q on partitions -> per-partition D."""
                for qc in range(2):
                    otn = otns[qc]
                    pav = psp.tile([128, 4 * 65], f32, tag="ps",
                                   name=f"pav_{t}_{qc}_{hg}")
                    for hl in range(4):
                        hx = hg * 4 + hl
                        for sc in range(2):
                            nc.tensor.matmul(
                                pav[:, hl * 65:(hl + 1) * 65],
                                pts[hx][:, sc * MTOK + qc * 128:
                                        sc * MTOK + (qc + 1) * 128],
                                vts[2 * ti + sc][:, hx * 66:hx * 66 + 65],
                                start=(sc == 0), stop=(sc == 1),
                                skip_group_check=True)
                    rcd = smp.tile([128, 4], f32, tag="rcd",
                                   name=f"rcd_{t}_{qc}_{hg}")
                    pav3 = pav[:].rearrange("p (h c) -> p h c", h=4)
                    nc.vector.reciprocal(rcd[:].unsqueeze(2),
                                         pav3[:, :, 64:65])
                    nc.vector.tensor_mul(
                        otn[:, hg * 256:(hg + 1) * 256].rearrange(
                            "p (h c) -> p h c", h=4),
                        pav3[:, :, 0:64],
                        rcd[:].unsqueeze(2).broadcast_to([128, 4, 64]))

            pend = None   # (t, otns) awaiting transpose + out-projection

            for tp in range(4):  # t-pairs: QK/V projection over 512 tokens
                ptok0 = tp * 2 * MTOK

                def emit_s(t, ti, hp, qk, pts):
                    """S^T + exp for head-pair hp of t. pss tiles come from a
                    dedicated PSUM pool so main-pool rotation never couples
                    PE matmuls to the ACT exp queue."""
                    psss = [psSp.tile([128, 2 * MTOK], f32, tag="pss",
                                      name=f"pss_{t}_{2 * hp + hh}")
                            for hh in range(2)]
                    # sc outer / hh inner: consecutive matmuls hit different
                    # PE row groups (ho = 0 vs 64) -> they run concurrently
                    for sc in range(2):
                        for hh in range(2):
                            ho = hh * 64
                            nc.tensor.matmul(
                                psss[hh][:, sc * MTOK:(sc + 1) * MTOK],
                                qk[4 + hp][ho:ho + 64,
                                           ti * MTOK + sc * 128:
                                           ti * MTOK + (sc + 1) * 128],
                                qk[hp][ho:ho + 64,
                                       ti * MTOK:(ti + 1) * MTOK],
                                start=True, stop=True,
                                tile_position=(ho, 0),
                                skip_group_check=True)
                    for hh in range(2):
                        h = 2 * hp + hh
                        pt = ptp.tile([128, 2 * MTOK], bf16, tag=f"pt{h}",
                                      name=f"pt{h}_{t}")
                        nc.scalar.activation(pt[:], psss[hh][:], Exp)
                        pts.append(pt)

                # ---- QK^T projection interleaved with S+exp of t0: the two
                # r-tiles head-pair hp needs (ri=hp, ri=hp+4), then its S
                # matmuls, so t0's exps stream on ACT across the whole
                # projection phase. t1's S is spread through the attention
                # tail below for the same reason. ----
                qk = [None] * 8
                pts0, pts1 = [], []

                def emit_qk(ri):
                    ps = psp.tile([128, 2 * MTOK], f32, tag="ps")
                    for fi in range(4):
                        nc.tensor.matmul(
                            ps[:],
                            wqkall[:, fi * 1024 + ri * 128:fi * 1024 + (ri + 1) * 128],
                            ytall[:, fi * TOK + ptok0:fi * TOK + ptok0 + 2 * MTOK],
                            start=(fi == 0), stop=(fi == 3))
                    qt = qkp.tile([128, 2 * MTOK], bf16, tag=f"qk{ri}",
                                  name=f"qk{ri}_{tp}")
                    # split the bias-evicts across ACT and DVE so neither
                    # queue delays the exps
                    if ri < 4:
                        nc.scalar.activation(qt[:], ps[:], Ident,
                                             bias=bqks[:, ri:ri + 1])
                    else:
                        nc.vector.tensor_scalar_add(qt[:], ps[:],
                                                    bqks[:, ri:ri + 1])
                    qk[ri] = qt

                # Q r-tiles first (they only need the Q halves of wqk, which
                # land first); each K r-tile then unlocks a head-pair's S
                for ri in range(4):
                    emit_qk(ri)
                for hp in range(4):
                    emit_qk(hp + 4)
                    emit_s(2 * tp, 0, hp, qk, pts0)
                # ---- V' projection: token-major (4 chunks of 128 tokens) ----
                vts = []
                for sc in range(4):
                    vt = vpool.tile([128, NH * 66], bf16, tag=f"v{sc}",
                                    name=f"v{sc}_{tp}")
                    # fi outer so the stationary (yT token chunk) is reused
                    # across both column halves -> half the LDWEIGHTS
                    psvs = [psp.tile([128, NH * 66 // 2], f32, tag="ps",
                                     name=f"psv_{tp}_{sc}_{h}")
                            for h in range(2)]
                    for fi in range(4):
                        for half in range(2):
                            c0 = half * (NH * 66 // 2)  # 264
                            nc.tensor.matmul(
                                psvs[half][:],
                                ytall[:, fi * TOK + ptok0 + sc * 128:fi * TOK + ptok0 + (sc + 1) * 128],
                                wvall[:, fi * NH * 66 + c0:fi * NH * 66 + c0 + NH * 66 // 2],
                                start=(fi == 0), stop=(fi == 3),
                                skip_group_check=True)
                    for half in range(2):
                        c0 = half * (NH * 66 // 2)
                        nc.vector.tensor_copy(vt[:, c0:c0 + NH * 66 // 2],
                                              psvs[half][:])
                    ones = vt[:].rearrange("p (h c) -> p h c", h=NH)[:, :, 64:65]
                    nc.gpsimd.memset(ones, 1.0)
                    vts.append(vt)

                # ---- attention tail: S(t1) head-pairs spread between AV/T/O
                # blocks so t1's exps overlap PE work instead of gating AV ----
                t0, t1 = 2 * tp, 2 * tp + 1
                otns0 = [onp.tile([128, E], bf16, tag=f"otn{qc}",
                                  name=f"otn{qc}_{t0}") for qc in range(2)]
                otns1 = [onp.tile([128, E], bf16, tag=f"otn{qc}",
                                  name=f"otn{qc}_{t1}") for qc in range(2)]
                if pend is not None:
                    otT = emit_transpose(*pend)
                emit_s(t1, 1, 0, qk, pts1)
                emit_av(t0, 0, pts0, vts, otns0, 0)
                emit_s(t1, 1, 1, qk, pts1)
                emit_av(t0, 0, pts0, vts, otns0, 1)
                emit_s(t1, 1, 2, qk, pts1)
                if pend is not None:
                    emit_outproj(pend[0], otT)
                emit_s(t1, 1, 3, qk, pts1)
                otT0 = emit_transpose(t0, otns0)
                emit_av(t1, 1, pts1, vts, otns1, 0)
                if tp < 3:
                    emit_outproj(t0, otT0)
                    emit_av(t1, 1, pts1, vts, otns1, 1)
                else:
                    # last t-pair: AV before O(t0) so the final normalize
                    # latency hides under the out-projection matmuls
                    emit_av(t1, 1, pts1, vts, otns1, 1)
                    emit_outproj(t0, otT0)
                pend = (t1, otns1)

            # drain the pipeline
            emit_outproj(pend[0], emit_transpose(*pend), last=True)

    if split_waits:
        _split_multiwaits(nc, mybir)
    return nc


def _host_prep(x, in_proj_w, in_proj_b, out_proj_w, out_proj_b):
    x = np.asarray(x, dtype=np.float32)
    in_proj_w = np.asarray(in_proj_w, dtype=np.float32)
    in_proj_b = np.asarray(in_proj_b, dtype=np.float32)
    out_proj_w = np.asarray(out_proj_w, dtype=np.float32)
    out_proj_b = np.asarray(out_proj_b, dtype=np.float32)

    # weights (replicated); fold the 1/sqrt(hd)=1/8 score scale into q rows
    wq = in_proj_w[:E] / 8.0
    wk = in_proj_w[E:2 * E]
    wv = in_proj_w[2 * E:]
    wqk = np.concatenate([wq, wk], 0).T.copy().astype(BF)        # (512, 1024)
    wv66 = np.zeros((E, NH * 66), dtype=np.float32)              # (512, 528)
    for h in range(NH):
        wv66[:, h * 66:h * 66 + 64] = wv[h * 64:(h + 1) * 64].T
    wv66 = wv66.astype(BF)
    wo = out_proj_w.T.copy().astype(BF)                          # (512, 512)
    bqk = np.concatenate([in_proj_b[:E] / 8.0, in_proj_b[E:2 * E]])
    bqk = bqk.reshape(8, 128).T.copy().astype(np.float32)        # (128, 8)
    beff = (out_proj_b + out_proj_w @ in_proj_b[2 * E:]).reshape(1, E)
    beff = beff.astype(np.float32)
    ident = np.eye(128, dtype=np.float32).astype(BF)             # (128, 128)

    # per-core token matrices: yT[f=(a,w1,w2), col=(tl, b, i, j)]
    # channel c = 64*cc + 8*tl + a  (t = 8*cc + tl)
    xv = x.reshape(B, HS, WIN, HS, WIN, NCORES, TL, WIN)  # b i w1 j w2 cc tl a
    yts = []
    for cc in range(NCORES):
        yt = xv[:, :, :, :, :, cc].transpose(6, 2, 4, 5, 0, 1, 3)
        yts.append(np.ascontiguousarray(yt).reshape(E, TOK).astype(BF))
    return yts, wqk, wv66, wo, bqk, beff, ident


def _in_maps(x, in_proj_w, in_proj_b, out_proj_w, out_proj_b):
    yts, wqk, wv66, wo, bqk, beff, ident = _host_prep(
        x, in_proj_w, in_proj_b, out_proj_w, out_proj_b)
    return [
        {"yT": yts[cc], "wqk": wqk, "wv": wv66, "wo": wo,
         "bqk": bqk, "beff": beff, "ident": ident}
        for cc in range(NCORES)
    ]


def kernel(x, in_proj_w, in_proj_b, out_proj_w, out_proj_b,
           window_size=8, nhead=8, **_unused):
    from concourse.bass_utils import run_bass_kernel_spmd

    in_maps = _in_maps(x, in_proj_w, in_proj_b, out_proj_w, out_proj_b)

    if "nc" not in _cache:
        _cache["nc"] = _build_module()
    nc = _cache["nc"]

    res = run_bass_kernel_spmd(nc, in_maps, core_ids=list(range(NCORES)))

    out = np.empty((B, N, E), dtype=np.float32)
    ov = out.reshape(B, HS, WIN, HS, WIN, E)  # b i w1 j w2 e
    for cc in range(NCORES):
        z = res.results[cc]["o"].reshape(TL, B, HS, HS, E)  # tl b i j e
        # t = 8*cc + tl -> w1 = cc, w2 = tl
        ov[:, :, cc, :, :, :] = z.transpose(1, 2, 3, 0, 4)
    return out


# revision 32
# speedup vs baseline: 1.0190x; 1.0190x over previous
"""Local2DAttention TRN2 kernel (nn_Local2DAttention_79207786873330).

Math (faithful to the reference's torch-bug semantics):
  x (16, 1024, 512) is window-blocked into M=256 "windows" (b, i, j) of 8x8
  spatial positions. A plain row-major reshape of each (E, 8, 8) block into
  (64, 512) scrambles channels/spatial into 64 tokens per window:
      y[m, t, e] = x[b, (i*8+w1)*32 + j*8 + w2, 8t+a],  e = a*64 + w1*8 + w2
  nn.MultiheadAttention (batch_first=False) then attends over the M=256 axis
  with the 64 t-positions as batch and 8 heads:
      per (t, h): S = Q K^T / 8 over 256x256, softmax, O = P V.

Sharding: the 64 t-positions split 8 per core (t = 8*cc + tl). Attention,
projections and output assembly are fully independent per t -> zero
cross-core communication. Weights are replicated.

Device pipeline per core (bf16 matmul operands, fp32 accumulation):
  yT (512, 2048)      - host-permuted token matrix, f-major (e x tokens)
  QK^T proj: PROJ^T[r, tok] = Wqk^T.T @ yT  (q rows pre-scaled by 1/8),
             bias-evicts split ACT (Identity + bias AP) / DVE (tensor_scalar)
  V    proj: V'[tok, 8x66]  = yT.T @ Wv''   (66-stride heads, ones col at 64)
  per (t, h):  S^T = K^T.T @ Q^T  - head pairs at tile_position (0,0)/(64,0)
               emitted row-group-alternating so they run CONCURRENTLY on the
               PE's 32x32 subarrays; exp on ACT (->bf16)
               AV swapped: stationary = P^T chunk, moving = V'[:, 65 cols]
                 -> O_u (q on PARTITIONS, 64 hd + D in col 64), 4 heads
                    packed per PSUM tile (128, 260)
               normalize: DVE reciprocal (128,4) + broadcast multiply
                 (per-partition D -> no PE broadcast matmul, no 1-lane recip)
  transpose O (PE, vs identity) -> O^T chunks -> evict to SBUF (DVE)
  out proj: Z = O^T.T @ Wout^T; Z += b_eff (DVE); DMA from SBUF (sync queue)

Schedule (the key to the 3.6x over the first working version):
  - per t-pair: QK r-tiles interleaved with S(t0) head-pairs, so t0's exps
    stream on ACT across the projection phase; S(t1) head-pairs spread
    between the AV/T/O blocks of the tail for the same reason. ACT is the
    attention-phase pacer - everything else must keep out of its way.
  - S-score PSUM tiles live in a dedicated 4-bank pool: main-pool rotation
    must never make a PE matmul wait on the ACT exp queue.
  - input DMAs: few large chunks with >=1KB per-partition runs (256-B runs
    measured ~4x slower), critical-first ordering, balanced across the
    gpsimd/sync/scalar queues; outputs all on sync (scalar triggers block
    exps in ACT's FIFO; gpsimd's end-drain with in-flight DMA costs ~3us).
"""
import sys
sys.path.insert(0, '/opt/trn_rl_repo')
import numpy as np
import ml_dtypes

BF = ml_dtypes.bfloat16

# problem constants (hardcoded per contract)
B, N, E = 16, 1024, 512
WIN = 8          # window_size
HS = 4           # hS = S // W,  S = 32
NH = 8           # heads
HD = 64          # head dim
NCORES = 8
TL = 8           # t-values per core
MTOK = 256       # windows (= B*HS*HS) = tokens per t
TOK = TL * MTOK  # tokens per core

_cache = {}


def _split_multiwaits(nc, mybir, limit=1):
    """This toolchain's walrus encodes at most one semaphore wait per
    instruction; hoist excess waits into preceding NoOps on the same engine."""
    n_split = 0
    for f in nc.m.functions:
        for blk in f.blocks:
            insts = blk.instructions
            out = []
            for inst in insts:
                si = inst.sync_info
                waits = list(si.on_wait) if (si is not None and si.on_wait) else []
                if len(waits) > limit:
                    excess, keep = waits[:-limit], waits[-limit:]
                    for w in excess:
                        nop = mybir.InstNoOp(
                            name=f"{inst.name}-wsplit{n_split}",
                            engine=inst.engine,
                            ins=[], outs=[],
                            sync_info=mybir.SyncInfo(on_wait=[w], on_update=[]),
                        )
                        out.append(nop)
                        n_split += 1
                    inst.sync_info = mybir.SyncInfo(
                        on_wait=keep, on_update=list(si.on_update or []))
                out.append(inst)
            if n_split:
                insts.clear()
                insts.extend(out)
    return n_split


def _build_module(split_waits=True):
    import concourse.bass as bass
    import concourse.mybir as mybir
    from concourse import tile

    f32 = mybir.dt.float32
    bf16 = mybir.dt.bfloat16
    Exp = mybir.ActivationFunctionType.Exp
    Ident = mybir.ActivationFunctionType.Identity

    nc = bass.Bass()
    YT = nc.dram_tensor("yT", [E, TOK], bf16, kind="ExternalInput")
    WQK = nc.dram_tensor("wqk", [E, 2 * E], bf16, kind="ExternalInput")
    WV = nc.dram_tensor("wv", [E, NH * 66], bf16, kind="ExternalInput")
    WO = nc.dram_tensor("wo", [E, E], bf16, kind="ExternalInput")
    BQK = nc.dram_tensor("bqk", [128, 8], f32, kind="ExternalInput")
    BEFF = nc.dram_tensor("beff", [1, E], f32, kind="ExternalInput")
    IDENT = nc.dram_tensor("ident", [128, 128], bf16, kind="ExternalInput")
    OUT = nc.dram_tensor("o", [TOK, E], f32, kind="ExternalOutput")

    with tile.TileContext(nc) as tc:
        with (
            tc.tile_pool(name="persist", bufs=1) as pers,
            tc.tile_pool(name="qk", bufs=2) as qkp,
            tc.tile_pool(name="v", bufs=2) as vpool,
            tc.tile_pool(name="pt", bufs=2) as ptp,
            tc.tile_pool(name="on", bufs=3) as onp,
            tc.tile_pool(name="ot", bufs=2) as otp,
            tc.tile_pool(name="sm", bufs=4) as smp,
            tc.tile_pool(name="z", bufs=3) as zp,
            tc.tile_pool(name="ps", bufs=4, space="PSUM") as psp,
            tc.tile_pool(name="psS", bufs=4, space="PSUM") as psSp,
        ):
            # ---- persistent loads -------------------------------------
            # Inputs live in single coalesced SBUF tiles (col-blocked by the
            # 128-row E chunk fi) so each load is ONE multi-block DMA -- the
            # head was trigger-serialization-bound with per-(fi,chunk) DMAs.
            # Order: the r-tile pairs the first S head-pairs need, and the
            # tp0 token slice, land first.
            bqks = pers.tile([128, 8], f32, tag="bqk")
            # fi-pair chunks keep per-partition runs at 1 KB (256-B runs of a
            # 4-block AP measured ~4x slower); Q halves land before K halves
            wqkall = pers.tile([128, 4 * 2 * E], bf16, tag="wqkall")
            wqk4 = wqkall[:].rearrange("p (f c) -> p f c", f=4)
            ytall = pers.tile([128, 4 * TOK], bf16, tag="ytall")
            yt4 = ytall[:].rearrange("p (f c) -> p f c", f=4)

            def load_wqk(rp, fp, eng):
                # quarter chunks (r-tile pair x fi pair, 128 KB) so the first
                # r-tiles are usable after ~128 KB per queue
                eng.dma_start(
                    wqk4[:, 2 * fp:2 * fp + 2, rp * 256:(rp + 1) * 256],
                    WQK[fp * 256:(fp + 1) * 256,
                        rp * 256:(rp + 1) * 256].rearrange(
                        "(f p) c -> p f c", f=2))

            def load_yt(tp, fp, eng):
                eng.dma_start(
                    yt4[:, 2 * fp:2 * fp + 2, tp * 512:(tp + 1) * 512],
                    YT[fp * 256:(fp + 1) * 256,
                       tp * 512:(tp + 1) * 512].rearrange(
                        "(f p) c -> p f c", f=2))

            # exactly ONE chunk per queue on the first-matmul critical path:
            # wqk r-tiles 0-1 (all fi) on gpsimd, yt tp0 halves on sync and
            # scalar; everything later arrives behind compute
            nc.gpsimd.dma_start(        # wqk r-tiles 0,1 all fi (256 KB)
                wqk4[:, :, 0:256],
                WQK[:, 0:256].rearrange("(f p) c -> p f c", f=4))
            load_yt(0, 0, nc.sync)
            load_yt(0, 1, nc.scalar)
            nc.gpsimd.dma_start(bqks[:], BQK[:])
            load_wqk(1, 0, nc.gpsimd)   # r-tiles 2,3
            load_wqk(1, 1, nc.sync)
            load_wqk(2, 0, nc.gpsimd)   # r-tiles 4,5
            load_wqk(2, 1, nc.sync)
            load_yt(1, 1, nc.scalar)
            load_wqk(3, 0, nc.gpsimd)   # r-tiles 6,7
            load_wqk(3, 1, nc.sync)
            load_yt(1, 0, nc.sync)
            for tp in (2, 3):
                load_yt(tp, 0, nc.sync)
                load_yt(tp, 1, nc.scalar)
            wvall = pers.tile([128, 4 * NH * 66], bf16, tag="wvall")
            wv4 = wvall[:].rearrange("p (f c) -> p f c", f=4)
            for fp in range(2):
                nc.gpsimd.dma_start(
                    wv4[:, 2 * fp:2 * fp + 2, :],
                    WV[fp * 256:(fp + 1) * 256, :].rearrange(
                        "(f p) c -> p f c", f=2))
            woall = pers.tile([128, 4 * E], bf16, tag="woall")
            wo4 = woall[:].rearrange("p (f c) -> p f c", f=4)
            for fp in range(2):
                nc.gpsimd.dma_start(
                    wo4[:, 2 * fp:2 * fp + 2, :],
                    WO[fp * 256:(fp + 1) * 256, :].rearrange(
                        "(f p) c -> p f c", f=2))
            beffb = pers.tile([128, E], f32, tag="beffb")
            nc.scalar.dma_start(
                beffb[:], BEFF[:].partition_broadcast(128).squeeze(1))
            ident = pers.tile([128, 128], bf16, tag="ident")
            nc.scalar.dma_start(ident[:], IDENT[:])



            def emit_transpose(t, otns):
                """O (q-part, E-free) -> O^T (E-part, m-free) via PE.
                qc-major bank packing: the out-proj of qc only needs its own
                bank's evict, so O(qc0) overlaps the qc1 transposes."""
                otT = otp.tile([128, 1024], bf16, tag="otT", name=f"otT_{t}")
                for qc in range(2):
                    psT = psp.tile([128, 512], bf16, tag="ps",
                                   name=f"psT_{t}_{qc}")
                    for ec in range(4):
                        nc.tensor.matmul(
                            psT[:, ec * 128:(ec + 1) * 128],
                            otns[qc][:, ec * 128:(ec + 1) * 128],
                            ident[:], is_transpose=True,
                            start=True, stop=True, skip_group_check=True)
                    nc.vector.tensor_copy(
                        otT[:, qc * 512:(qc + 1) * 512], psT[:])
                return otT

            def emit_outproj(t, otT, last=False):
                tok0 = t * MTOK
                for qc in range(2):
                    if last and qc == 1:
                        # final store: column-half accumulation groups so the
                        # DVE add + DMA of half 0 overlap half 1's matmuls;
                        # the two stores go on different queues (ACT's FIFO
                        # is empty at drain time, so scalar is safe here)
                        zt = zp.tile([128, E], f32, tag="zt",
                                     name=f"zt_{t}_{qc}")
                        for ch in range(2):
                            pszh = psp.tile([128, 256], f32, tag="ps",
                                            name=f"pszh_{t}_{ch}")
                            for ec in range(4):
                                nc.tensor.matmul(
                                    pszh[:],
                                    otT[:, qc * 512 + ec * 128:
                                        qc * 512 + (ec + 1) * 128],
                                    woall[:, ec * E + ch * 256:
                                           ec * E + (ch + 1) * 256],
                                    start=(ec == 0), stop=(ec == 3))
                            nc.vector.tensor_add(
                                zt[:, ch * 256:(ch + 1) * 256], pszh[:],
                                beffb[:, ch * 256:(ch + 1) * 256])
                            eng = nc.sync if ch == 0 else nc.scalar
                            eng.dma_start(
                                OUT[tok0 + qc * 128:tok0 + (qc + 1) * 128,
                                    ch * 256:(ch + 1) * 256],
                                zt[:, ch * 256:(ch + 1) * 256])
                        continue
                    psz = psp.tile([128, E], f32, tag="ps",
                                   name=f"psz_{t}_{qc}")
                    for ec in range(4):
                        nc.tensor.matmul(
                            psz[:],
                            otT[:, qc * 512 + ec * 128:qc * 512 + (ec + 1) * 128],
                            woall[:, ec * E:(ec + 1) * E],
                            start=(ec == 0), stop=(ec == 3))
                    zt = zp.tile([128, E], f32, tag="zt", name=f"zt_{t}_{qc}")
                    # all outputs on sync: scalar triggers would block exps in
                    # ACT's FIFO, and a gpsimd queue with in-flight DMA costs
                    # ~3us in the end-of-kernel drain
                    nc.vector.tensor_add(zt[:], psz[:], beffb[:])
                    nc.sync.dma_start(
                        OUT[tok0 + qc * 128:tok0 + (qc + 1) * 128, :],
                        zt[:])

            def emit_av(t, ti, pts, vts, otns, hg):
                """AV (swapped) + normalize for one head-group of t, both
                query chunks. q on partitions -> per-partition D."""
                for qc in range(2):
                    otn = otns[qc]
                    pav = psp.tile([128, 4 * 65], f32, tag="ps",
                                   name=f"pav_{t}_{qc}_{hg}")
                    for hl in range(4):
                        hx = hg * 4 + hl
                        for sc in range(2):
                            nc.tensor.matmul(
                                pav[:, hl * 65:(hl + 1) * 65],
                                pts[hx][:, sc * MTOK + qc * 128:
                                        sc * MTOK + (qc + 1) * 128],
                                vts[2 * ti + sc][:, hx * 66:hx * 66 + 65],
                                start=(sc == 0), stop=(sc == 1),
                                skip_group_check=True)
                    rcd = smp.tile([128, 4], f32, tag="rcd",
                                   name=f"rcd_{t}_{qc}_{hg}")
                    pav3 = pav[:].rearrange("p (h c) -> p h c", h=4)
                    nc.vector.reciprocal(rcd[:].unsqueeze(2),
                                         pav3[:, :, 64:65])
                    nc.vector.tensor_mul(
                        otn[:, hg * 256:(hg + 1) * 256].rearrange(
                            "p (h c) -> p h c", h=4),
                        pav3[:, :, 0:64],
                        rcd[:].unsqueeze(2).broadcast_to([128, 4, 64]))

            pend = None   # (t, otns) awaiting transpose + out-projection

            for tp in range(4):  # t-pairs: QK/V projection over 512 tokens
                ptok0 = tp * 2 * MTOK

                def emit_s(t, ti, hp, qk, pts):
                    """S^T + exp for head-pair hp of t. pss tiles come from a
                    dedicated PSUM pool so main-pool rotation never couples
                    PE matmuls to the ACT exp queue."""
                    psss = [psSp.tile([128, 2 * MTOK], f32, tag="pss",
                                      name=f"pss_{t}_{2 * hp + hh}")
                            for hh in range(2)]
                    # sc outer / hh inner: consecutive matmuls hit different
                    # PE row groups (ho = 0 vs 64) -> they run concurrently
                    for sc in range(2):
                        for hh in range(2):
                            ho = hh * 64
                            nc.tensor.matmul(
                                psss[hh][:, sc * MTOK:(sc + 1) * MTOK],
                                qk[4 + hp][ho:ho + 64,
                                           ti * MTOK + sc * 128:
                                           ti * MTOK + (sc + 1) * 128],
                                qk[hp][ho:ho + 64,
                                       ti * MTOK:(ti + 1) * MTOK],
                                start=True, stop=True,
                                tile_position=(ho, 0),
                                skip_group_check=True)
                    for hh in range(2):
                        h = 2 * hp + hh
                        pt = ptp.tile([128, 2 * MTOK], bf16, tag=f"pt{h}",
                                      name=f"pt{h}_{t}")
                        nc.scalar.activation(pt[:], psss[hh][:], Exp)
                        pts.append(pt)

                # ---- QK^T projection interleaved with S+exp of t0: the two
                # r-tiles head-pair hp needs (ri=hp, ri=hp+4), then its S
                # matmuls, so t0's exps stream on ACT across the whole
                # projection phase. t1's S is spread through the attention
                # tail below for the same reason. ----
                qk = [None] * 8
                pts0, pts1 = [], []

                def emit_qk(ri):
                    ps = psp.tile([128, 2 * MTOK], f32, tag="ps")
                    for fi in range(4):
                        nc.tensor.matmul(
                            ps[:],
                            wqkall[:, fi * 1024 + ri * 128:fi * 1024 + (ri + 1) * 128],
                            ytall[:, fi * TOK + ptok0:fi * TOK + ptok0 + 2 * MTOK],
                            start=(fi == 0), stop=(fi == 3))
                    qt = qkp.tile([128, 2 * MTOK], bf16, tag=f"qk{ri}",
                                  name=f"qk{ri}_{tp}")
                    # split the bias-evicts across ACT and DVE so neither
                    # queue delays the exps
                    if ri < 4:
                        nc.scalar.activation(qt[:], ps[:], Ident,
                                             bias=bqks[:, ri:ri + 1])
                    else:
                        nc.vector.tensor_scalar_add(qt[:], ps[:],
                                                    bqks[:, ri:ri + 1])
                    qk[ri] = qt

                # Q r-tiles first (they only need the Q halves of wqk, which
                # land first); each K r-tile then unlocks a head-pair's S
                for ri in range(4):
                    emit_qk(ri)
                for hp in range(4):
                    emit_qk(hp + 4)
                    emit_s(2 * tp, 0, hp, qk, pts0)
                # ---- V' projection: token-major (4 chunks of 128 tokens) ----
                vts = []
                for sc in range(4):
                    vt = vpool.tile([128, NH * 66], bf16, tag=f"v{sc}",
                                    name=f"v{sc}_{tp}")
                    # fi outer so the stationary (yT token chunk) is reused
                    # across both column halves -> half the LDWEIGHTS
                    psvs = [psp.tile([128, NH * 66 // 2], f32, tag="ps",
                                     name=f"psv_{tp}_{sc}_{h}")
                            for h in range(2)]
                    for fi in range(4):
                        for half in range(2):
                            c0 = half * (NH * 66 // 2)  # 264
                            nc.tensor.matmul(
                                psvs[half][:],
                                ytall[:, fi * TOK + ptok0 + sc * 128:fi * TOK + ptok0 + (sc + 1) * 128],
                                wvall[:, fi * NH * 66 + c0:fi * NH * 66 + c0 + NH * 66 // 2],
                                start=(fi == 0), stop=(fi == 3),
                                skip_group_check=True)
                    for half in range(2):
                        c0 = half * (NH * 66 // 2)
                        nc.vector.tensor_copy(vt[:, c0:c0 + NH * 66 // 2],
                                              psvs[half][:])
                    ones = vt[:].rearrange("p (h c) -> p h c", h=NH)[:, :, 64:65]
                    nc.gpsimd.memset(ones, 1.0)
                    vts.append(vt)

                # ---- attention tail: S(t1) head-pairs spread between AV/T/O
                # blocks so t1's exps overlap PE work instead of gating AV ----
                t0, t1 = 2 * tp, 2 * tp + 1
                otns0 = [onp.tile([128, E], bf16, tag=f"otn{qc}",
                                  name=f"otn{qc}_{t0}") for qc in range(2)]
                otns1 = [onp.tile([128, E], bf16, tag=f"otn{qc}",
                                  name=f"otn{qc}_{t1}") for qc in range(2)]
                if pend is not None:
                    otT = emit_transpose(*pend)
                emit_s(t1, 1, 0, qk, pts1)
                emit_av(t0, 0, pts0, vts, otns0, 0)
                emit_s(t1, 1, 1, qk, pts1)
                emit_av(t0, 0, pts0, vts, otns0, 1)
                emit_s(t1, 1, 2, qk, pts1)
                if pend is not None:
                    emit_outproj(pend[0], otT)
                emit_s(t1, 1, 3, qk, pts1)
                otT0 = emit_transpose(t0, otns0)
                emit_av(t1, 1, pts1, vts, otns1, 0)
                if tp < 3:
                    emit_outproj(t0, otT0)
                    emit_av(t1, 1, pts1, vts, otns1, 1)
                else:
                    # last t-pair: AV before O(t0) so the final normalize
                    # latency hides under the out-projection matmuls
                    emit_av(t1, 1, pts1, vts, otns1, 1)
                    emit_outproj(t0, otT0)
                pend = (t1, otns1)

            # drain the pipeline
            emit_outproj(pend[0], emit_transpose(*pend), last=True)

    if split_waits:
        _split_multiwaits(nc, mybir)
    return nc


def _host_prep(x, in_proj_w, in_proj_b, out_proj_w, out_proj_b):
    x = np.asarray(x, dtype=np.float32)
    in_proj_w = np.asarray(in_proj_w, dtype=np.float32)
    in_proj_b = np.asarray(in_proj_b, dtype=np.float32)
    out_proj_w = np.asarray(out_proj_w, dtype=np.float32)
    out_proj_b = np.asarray(out_proj_b, dtype=np.float32)

    # weights (replicated); fold the 1/sqrt(hd)=1/8 score scale into q rows
    wq = in_proj_w[:E] / 8.0
    wk = in_proj_w[E:2 * E]
    wv = in_proj_w[2 * E:]
    wqk = np.concatenate([wq, wk], 0).T.copy().astype(BF)        # (512, 1024)
    wv66 = np.zeros((E, NH * 66), dtype=np.float32)              # (512, 528)
    for h in range(NH):
        wv66[:, h * 66:h * 66 + 64] = wv[h * 64:(h + 1) * 64].T
    wv66 = wv66.astype(BF)
    wo = out_proj_w.T.copy().astype(BF)                          # (512, 512)
    bqk = np.concatenate([in_proj_b[:E] / 8.0, in_proj_b[E:2 * E]])
    bqk = bqk.reshape(8, 128).T.copy().astype(np.float32)        # (128, 8)
    beff = (out_proj_b + out_proj_w @ in_proj_b[2 * E:]).reshape(1, E)
    beff = beff.astype(np.float32)
    ident = np.eye(128, dtype=np.float32).astype(BF)             # (128, 128)

    # per-core token matrices: yT[f=(a,w1,w2), col=(tl, b, i, j)]
    # channel c = 64*cc + 8*tl + a  (t = 8*cc + tl)
    xv = x.reshape(B, HS, WIN, HS, WIN, NCORES, TL, WIN)  # b i w1 j w2 cc tl a
    yts = []
    for cc in range(NCORES):
        yt = xv[:, :, :, :, :, cc].transpose(6, 2, 4, 5, 0, 1, 3)
        yts.append(np.ascontiguousarray(yt).reshape(E, TOK).astype(BF))
    return yts, wqk, wv66, wo, bqk, beff, ident


def _in_maps(x, in_proj_w, in_proj_b, out_proj_w, out_proj_b):
    yts, wqk, wv66, wo, bqk, beff, ident = _host_prep(
        x, in_proj_w, in_proj_b, out_proj_w, out_proj_b)
    return [
        {"yT": yts[cc], "wqk": wqk, "wv": wv66, "wo": wo,
         "bqk": bqk, "beff": beff, "ident": ident}
        for cc in range(NCORES)
    ]


def kernel(x, in_proj_w, in_proj_b, out_proj_w, out_proj_b,
           window_size=8, nhead=8, **_unused):
    from concourse.bass_utils import run_bass_kernel_spmd

    in_maps = _in_maps(x, in_proj_w, in_proj_b, out_proj_w, out_proj_b)

    if "nc" not in _cache:
        _cache["nc"] = _build_module()
    nc = _cache["nc"]

    res = run_bass_kernel_spmd(nc, in_maps, core_ids=list(range(NCORES)))

    out = np.empty((B, N, E), dtype=np.float32)
    ov = out.reshape(B, HS, WIN, HS, WIN, E)  # b i w1 j w2 e
    for cc in range(NCORES):
        z = res.results[cc]["o"].reshape(TL, B, HS, HS, E)  # tl b i j e
        # t = 8*cc + tl -> w1 = cc, w2 = tl
        ov[:, :, cc, :, :, :] = z.transpose(1, 2, 3, 0, 4)
    return out
